# revision 11
# baseline (speedup 1.0000x reference)
"""Trainium2 Bass kernel: FADEv4 retrieval-kNN head (nn_FADEv4_7026566496861).

Math (per image n):
    cls  = l2norm(mean_s(x_support_cls[n]))          # [1,D]
    q    = l2norm(x_query[n])                        # [Tq,D]
    s    = l2norm(x_support[n])                      # [Ts,D]
    sim  = q @ s.T                                   # [Tq,Ts]
    dmin = 1 - max_ts(sim); idx = argmax_ts(sim)
    pred = sigmoid(q@W1 + s[idx]@W2 + cls@W3 + b)
    out0 = (pred*dmin).reshape(N,1,37,37); out1 = pred.reshape(N,1,37,37)

Sharding: data-parallel over N=16 images -> 8 cores x 2 images, no collectives.

v2 design (vs v1 which used PE transposes + bf16 matmuls):
  * Normalization pipeline per 128-token tile:
      ACT Square+accum (ssq) -> ACT Sqrt(scale=1/768) (denom = |x|*sqrt(1/768))
      -> GPSIMD normalize_recip (divide + cast to bf16; so normalized rows are
      sqrt(768)*x_hat) -> XBAR dma_start_transpose (bf16, [tok,768]->[128,6,tok])
      -> ACT copy-cast bf16->fp8e4.
    No PE transposes, no PSUM->SBUF copies.
  * Big matmul in fp8e4 with MatmulPerfMode.DoubleRow (2 contraction chunks
    per instruction, 2x PE throughput): sim_psum = 768 * cosine.
  * W1 appended as an extra support column (p1 = 768*q_hat@W1 falls out of the
    matmul); W2 as an extra query column (p2 row staged to DRAM, gathered per
    query row by indirect DMA).
  * Supports processed in NG=3 column groups (2048/2048/1408 incl W1) with
    [128,2048] f32 PSUM tiles (4 banks x 2 bufs); DVE max8 + max_index per
    (m-block, group).
  * Combine/argmax/head fully batched per image ([128, MB]-shaped ops).
"""

import math
import os

import numpy as np

import concourse.bass as bass
import concourse.mybir as mybir
import concourse.tile as tile
from concourse import bacc
from concourse.bass import ds, IndirectOffsetOnAxis
from concourse.bass_utils import run_bass_kernel_spmd

F32 = mybir.dt.float32
BF16 = mybir.dt.bfloat16
FP8 = mybir.dt.float8e4
U32 = mybir.dt.uint32
AX = mybir.AxisListType
OP = mybir.AluOpType
ACTF = mybir.ActivationFunctionType
PERF = mybir.MatmulPerfMode

N_FULL, TQ, TS, S, D = 16, 1369, 5476, 4, 768
SIDE = 37
KC = D // 128                       # 6 contraction chunks of 128
SSCALE = float(np.float32(1.0 / 768.0))   # sim_psum = cos / SSCALE
WSCALE = float(1.0 / math.sqrt(1.0 / 768.0))  # scale for W1/W2 columns
W2COL = 1376                        # W2 column in qT
TQE = W2COL + 1                     # 1377 qT columns
QT_PITCH = 1392                     # qTq k-chunk pitch; %16==0 for dual-fp8 LDW
MB = (TQE + 127) // 128             # 11 m-blocks (last: 97 cols, 89 real)
GW = 2048                           # support group width (4 PSUM banks)
NG = 3                              # groups: 2048, 2048, 1408
G2W = 1408                          # g2 sTq extent (512+512+384 MM chunks)
G2REAL = 1380                       # real supports in g2; W1 at col 1380
G2TB = 1392                         # extent written into sTb for g2
QTB_W = 1376                        # qTb extent (1280 + 96 padded edge)
P2D_LEN = TS                        # staged p2 covers exactly the real supports
NEG = -1.0e30
W_ALL = MB * NG * 8                 # 264 combine width

N_CORES = 8
PER_CORE = N_FULL // N_CORES

USE_DR = os.environ.get("FADE_DR", "1") == "1"   # DoubleRow fp8; else bf16
MM_DT = FP8 if USE_DR else BF16


def q_tiles():
    t = [(i * 128, 128, 128) for i in range(10)]
    t.append((1280, 89, 96))
    return t


def s_tiles(g):
    base = g * GW
    if g < 2:
        return [(base + i * 128, 128, 128) for i in range(16)]
    t = [(base + i * 128, 128, 128) for i in range(10)]
    t.append((base + 1280, 100, 112))
    return t


def _mm_chunks(g):
    if g < NG - 1:
        return [(0, 512), (512, 512), (1024, 512), (1536, 512)]
    return [(0, 512), (512, 512), (1024, 384)]


def _emit_image(nc, pools, consts, aps, n, stage=99):
    (img_pool, qtb_pool, stb_pool, stq_pool, scratch, cls_pool, p2_pool,
     psum_mm) = pools
    (w1f, w2f, w3, bh, cbase_f) = consts
    (x_query, x_support, x_cls, p2d_list, c3d_list, out0, out1) = aps

    # ---- cls head scalar: c3b = (cls_hat . W3) + b, broadcast to [128,1] ----
    clsbig = cls_pool.tile([1, S * D], F32, tag="clsbig")
    nc.sync.dma_start(out=clsbig[:, :], in_=x_cls[n])
    clsum = cls_pool.tile([1, D], F32, tag="clsum")
    nc.vector.tensor_add(clsum[:, :], clsbig[:, 0:D], clsbig[:, D:2 * D])
    nc.vector.tensor_add(clsum[:, :], clsum[:, :], clsbig[:, 2 * D:3 * D])
    nc.vector.tensor_add(clsum[:, :], clsum[:, :], clsbig[:, 3 * D:4 * D])
    sc3 = cls_pool.tile([1, D], F32, tag="sc3")
    ss3 = cls_pool.tile([1, 8], F32, tag="ss3")
    nc.vector.tensor_mul(sc3[:, :], clsum[:, :], clsum[:, :])
    nc.vector.tensor_reduce(out=ss3[:, 0:1], in_=sc3[:, :], axis=AX.X, op=OP.add)
    nc.vector.tensor_mul(sc3[:, :], clsum[:, :], w3[:, :])
    nc.vector.tensor_reduce(out=ss3[:, 1:2], in_=sc3[:, :], axis=AX.X, op=OP.add)
    nc.scalar.sqrt(ss3[:, 2:3], ss3[:, 0:1])
    nc.vector.reciprocal(ss3[:, 3:4], ss3[:, 2:3])
    nc.vector.tensor_mul(ss3[:, 4:5], ss3[:, 1:2], ss3[:, 3:4])
    nc.vector.tensor_add(ss3[:, 5:6], ss3[:, 4:5], bh[:, 0:1])
    nc.sync.dma_start(out=c3d_list[n][:, :], in_=ss3[0:1, 5:6])
    c3b = img_pool.tile([128, 1], F32, tag="c3b")
    nc.sync.dma_start(out=c3b[:, :], in_=c3d_list[n][:, :].to_broadcast((128, 1)))

    # ---- normalize + XBAR-transpose one token tile into dstb (bf16) ----
    tp_count = [0]

    def build_tile(src_row0, tok0, rows, rows_pad, dstb, dst_off):
        raw = scratch.tile([128, D], F32, tag="raw")
        if rows < rows_pad:
            nc.vector.memset(raw[:, :], 1.0)
        nc.sync.dma_start(out=raw[:rows, :], in_=src_row0[ds(tok0, rows), :])
        sqd = scratch.tile([128, D], BF16, tag="sqd")
        ssq = scratch.tile([128, 1], F32, tag="ssq")
        nc.scalar.activation(
            sqd[:rows_pad, :], raw[:rows_pad, :], ACTF.Square,
            accum_out=ssq[:rows_pad, :],
        )
        den = scratch.tile([128, 1], F32, tag="den")
        nc.scalar.activation(den[:rows_pad, :], ssq[:rows_pad, :], ACTF.Sqrt,
                             scale=SSCALE)
        nmb = scratch.tile([128, D], BF16, tag="nmb")
        nc.gpsimd.normalize_recip(nmb[:rows_pad, :], raw[:rows_pad, :],
                                  den[:rows_pad, :])
        eng = nc.sync if (tp_count[0] % 2 == 0) else nc.scalar
        tp_count[0] += 1
        eng.dma_start_transpose(
            out=dstb[:, :, ds(dst_off, rows_pad)], in_=nmb[:rows_pad, :]
        )

    # ---- build qT (bf16 staged, then cast to fp8) ----
    qTb = qtb_pool.tile([128, KC, QTB_W], BF16, tag="qTb")
    for (tok0, rows, rows_pad) in q_tiles():
        build_tile(x_query[n], tok0, rows, rows_pad, qTb, tok0)
    qTq = img_pool.tile([128, KC, QT_PITCH], MM_DT, tag="qTq")
    for (c0, c1) in ((0, 512), (512, 1024), (1024, QTB_W)):
        nc.scalar.copy(qTq[:, :, c0:c1], qTb[:, :, c0:c1])
    nc.vector.memset(qTq[:, :, TQ:W2COL], 0)
    nc.vector.tensor_copy(
        qTq[:, :, W2COL:W2COL + 1].rearrange("p a b -> p (a b)"), w2f[:, :]
    )

    if stage < 2:
        z0 = scratch.tile([128, MB], F32, tag="z0")
        nc.vector.memset(z0[:, :], 0.25)
        for m in range(MB):
            mreal = 128 if m < MB - 1 else TQ - 128 * (MB - 1)
            nc.sync.dma_start(out=out1[n, ds(m * 128, mreal)], in_=z0[:mreal, m:m + 1])
            nc.sync.dma_start(out=out0[n, ds(m * 128, mreal)], in_=z0[:mreal, m:m + 1])
        return

    # ---- per-image state ----
    Mc8 = img_pool.tile([128, MB, NG, 8], F32, tag="Mc8")
    Ic8 = img_pool.tile([128, MB, NG, 8], U32, tag="Ic8")
    p1_all = img_pool.tile([128, MB], F32, tag="p1")
    # rows >= 97 of the last m-block are never written by max8; give them
    # defined values so the batched combine stays finite (clamp handles the
    # all-equal one-hot sum).
    nc.vector.memset(Mc8[:, :, :, :].rearrange("p a b c -> p (a b c)"), NEG)
    nc.vector.memset(Ic8[:, :, :, :].rearrange("p a b c -> p (a b c)"), 0)
    nc.vector.memset(p1_all[:, :], 0)

    def build_group(g):
        sTb = stb_pool.tile([128, KC, GW], BF16, tag="sTb")
        for (tok0, rows, rows_pad) in s_tiles(g):
            build_tile(x_support[n], tok0, rows, rows_pad, sTb, tok0 - g * GW)
        sTq = stq_pool.tile([128, KC, GW], MM_DT, tag="sTq")
        if g < NG - 1:
            for (c0, c1) in ((0, 512), (512, 1024), (1024, 1536), (1536, 2048)):
                nc.scalar.copy(sTq[:, :, c0:c1], sTb[:, :, c0:c1])
        else:
            for (c0, c1) in ((0, 512), (512, 1024), (1024, G2TB)):
                nc.scalar.copy(sTq[:, :, c0:c1], sTb[:, :, c0:c1])
            nc.vector.memset(sTq[:, :, G2TB:G2W], 0)
            nc.vector.tensor_copy(
                sTq[:, :, G2REAL:G2REAL + 1].rearrange("p a b -> p (a b)"),
                w1f[:, :],
            )
        return sTq

    def mm_group(g, sTq):
        gn = GW if g < NG - 1 else G2REAL
        for m in range(MB):
            mcols = 128 if m < MB - 1 else TQE - 128 * (MB - 1)
            bp = psum_mm.tile([128, GW], F32, tag="bp")
            for (c0, cw) in _mm_chunks(g):
                if USE_DR:
                    for k2 in range(KC // 2):
                        nc.tensor.matmul(
                            bp[:mcols, ds(c0, cw)],
                            lhsT=qTq[:, 2 * k2:2 * k2 + 2, ds(m * 128, mcols)],
                            rhs=sTq[:, 2 * k2:2 * k2 + 2, ds(c0, cw)],
                            start=(k2 == 0), stop=(k2 == KC // 2 - 1),
                            perf_mode=PERF.DoubleRow,
                        )
                else:
                    for k in range(KC):
                        nc.tensor.matmul(
                            bp[:mcols, ds(c0, cw)],
                            lhsT=qTq[:, k, ds(m * 128, mcols)],
                            rhs=sTq[:, k, ds(c0, cw)],
                            start=(k == 0), stop=(k == KC - 1),
                        )
            nc.vector.max(Mc8[:mcols, m, g, :], bp[:mcols, :gn])
            nc.vector.max_index(
                Ic8[:mcols, m, g, :], Mc8[:mcols, m, g, :], bp[:mcols, :gn]
            )
            if g == NG - 1:
                nc.scalar.copy(p1_all[:mcols, m:m + 1],
                               bp[:mcols, G2REAL:G2REAL + 1])
            if m == MB - 1:
                p2row = p2_pool.tile([1, GW], F32, tag="p2row")
                nc.scalar.copy(p2row[0:1, :gn], bp[96:97, :gn])
                nc.sync.dma_start(
                    out=p2d_list[n][ds(g * GW, gn), 0], in_=p2row[0:1, :gn]
                )

    # software-pipelined: build group g+1 before the matmuls of group g
    sT0 = build_group(0)
    sT1 = build_group(1)
    mm_group(0, sT0)
    sT2 = build_group(2)
    mm_group(1, sT1)
    mm_group(2, sT2)

    if stage < 3:
        z0 = scratch.tile([128, MB], F32, tag="z0")
        nc.vector.tensor_copy(z0[:, :], Mc8[:, :, 0, 0])
        for m in range(MB):
            mreal = 128 if m < MB - 1 else TQ - 128 * (MB - 1)
            nc.sync.dma_start(out=out1[n, ds(m * 128, mreal)], in_=z0[:mreal, m:m + 1])
            nc.sync.dma_start(out=out0[n, ds(m * 128, mreal)], in_=p1_all[:mreal, m:m + 1])
        return

    # ---- batched combine: global max / argmax per m-block ----
    mcA = Mc8[:, :, :, :].rearrange("p a b c -> p a (b c)")   # [128, MB, 24]
    icA = Ic8[:, :, :, :].rearrange("p a b c -> p (a b c)")   # [128, 264]
    gmax = img_pool.tile([128, MB], F32, tag="gmax")
    nc.vector.tensor_reduce(out=gmax[:, :], in_=mcA, axis=AX.X, op=OP.max)
    oneh = img_pool.tile([128, MB, NG * 8], F32, tag="oneh")
    for m in range(MB):
        nc.vector.tensor_scalar(
            out=oneh[:, m, :], in0=mcA[:, m, :], scalar1=gmax[:, m:m + 1],
            scalar2=None, op0=OP.is_equal,
        )
    icf = img_pool.tile([128, MB, NG * 8], F32, tag="icf")
    icfF = icf[:, :, :].rearrange("p a b -> p (a b)")
    nc.vector.tensor_copy(icfF, icA)
    nc.vector.tensor_add(icfF, icfF, cbase_f[:, :])
    nc.vector.tensor_mul(icfF, icfF,
                         oneh[:, :, :].rearrange("p a b -> p (a b)"))
    gidxf = img_pool.tile([128, MB], F32, tag="gidxf")
    nc.vector.tensor_reduce(
        out=gidxf[:, :], in_=icf[:, :, :], axis=AX.X, op=OP.add,
    )
    nc.vector.tensor_scalar_min(gidxf[:, :], gidxf[:, :], float(TS - 1))
    gidx = img_pool.tile([128, MB], U32, tag="gidx")
    nc.vector.tensor_copy(gidx[:, :], gidxf[:, :])
    dmin_all = img_pool.tile([128, MB], F32, tag="dmin")
    nc.scalar.activation(dmin_all[:, :], gmax[:, :], ACTF.Copy,
                         bias=1.0, scale=-SSCALE)

    # ---- p2 gather + batched head ----
    p2g = img_pool.tile([128, MB], F32, tag="p2g")
    for m in range(MB):
        nc.gpsimd.indirect_dma_start(
            out=p2g[:, m:m + 1], out_offset=None, in_=p2d_list[n][:, :],
            in_offset=IndirectOffsetOnAxis(ap=gidx[:, m:m + 1], axis=0),
        )
    lg = img_pool.tile([128, MB], F32, tag="lg")
    nc.vector.tensor_add(lg[:, :], p1_all[:, :], p2g[:, :])
    pred = img_pool.tile([128, MB], F32, tag="pred")
    nc.scalar.activation(pred[:, :], lg[:, :], ACTF.Sigmoid,
                         bias=c3b[:, 0:1], scale=SSCALE)
    o0 = img_pool.tile([128, MB], F32, tag="o0")
    nc.vector.tensor_mul(o0[:, :], pred[:, :], dmin_all[:, :])
    for m in range(MB):
        mreal = 128 if m < MB - 1 else TQ - 128 * (MB - 1)
        nc.sync.dma_start(out=out1[n, ds(m * 128, mreal)], in_=pred[:mreal, m:m + 1])
        nc.sync.dma_start(out=out0[n, ds(m * 128, mreal)], in_=o0[:mreal, m:m + 1])


def build_program(per_core=PER_CORE, stage=99):
    nc = bacc.Bacc("TRN2", target_bir_lowering=False, debug=False)
    x_query = nc.dram_tensor("x_query", [per_core, TQ, D], F32, kind="ExternalInput").ap()
    x_support = nc.dram_tensor("x_support", [per_core, TS, D], F32, kind="ExternalInput").ap()
    x_cls = nc.dram_tensor("x_support_cls", [per_core, S * D], F32, kind="ExternalInput").ap()
    w_head = nc.dram_tensor("W_head", [3 * D, 1], F32, kind="ExternalInput").ap()
    b_head = nc.dram_tensor("b_head", [1, 1], F32, kind="ExternalInput").ap()
    out0 = nc.dram_tensor("out0", [per_core, TQ], F32, kind="ExternalOutput").ap()
    out1 = nc.dram_tensor("out1", [per_core, TQ], F32, kind="ExternalOutput").ap()
    p2d_list = [
        nc.dram_tensor(f"p2d_{n}", [P2D_LEN, 1], F32).ap() for n in range(per_core)
    ]
    c3d_list = [
        nc.dram_tensor(f"c3d_{n}", [1, 1], F32).ap() for n in range(per_core)
    ]

    with tile.TileContext(nc) as tc:
        with tc.tile_pool(name="img", bufs=2) as img_pool, \
             tc.tile_pool(name="qtb", bufs=1) as qtb_pool, \
             tc.tile_pool(name="stb", bufs=2) as stb_pool, \
             tc.tile_pool(name="stq", bufs=2) as stq_pool, \
             tc.tile_pool(name="scratch", bufs=4) as scratch, \
             tc.tile_pool(name="cls", bufs=1) as cls_pool, \
             tc.tile_pool(name="p2", bufs=2) as p2_pool, \
             tc.tile_pool(name="const", bufs=1) as const_pool, \
             tc.tile_pool(name="psum_mm", bufs=2, space="PSUM") as psum_mm:

            # constants
            w1s = const_pool.tile([128, KC], F32)
            w2s = const_pool.tile([128, KC], F32)
            w3 = const_pool.tile([1, D], F32)
            bh = const_pool.tile([1, 1], F32)
            for k in range(KC):
                nc.sync.dma_start(out=w1s[:, k:k + 1], in_=w_head[ds(128 * k, 128), :])
                nc.sync.dma_start(out=w2s[:, k:k + 1], in_=w_head[ds(D + 128 * k, 128), :])
            nc.sync.dma_start(out=w3[0:1, :], in_=w_head[ds(2 * D, D), :])
            nc.sync.dma_start(out=bh[:, :], in_=b_head[:, :])
            w1f = const_pool.tile([128, KC], MM_DT)
            w2f = const_pool.tile([128, KC], MM_DT)
            nc.scalar.activation(w1f[:, :], w1s[:, :], ACTF.Copy, scale=WSCALE)
            nc.scalar.activation(w2f[:, :], w2s[:, :], ACTF.Copy, scale=WSCALE)
            cbase_u = const_pool.tile([128, W_ALL], U32)
            cbase_f = const_pool.tile([128, W_ALL], F32)
            nc.gpsimd.iota(cbase_u[:, :], pattern=[[0, MB], [GW, NG], [0, 8]],
                           base=0, channel_multiplier=0)
            nc.vector.tensor_copy(cbase_f[:, :], cbase_u[:, :])

            pools = (img_pool, qtb_pool, stb_pool, stq_pool, scratch, cls_pool,
                     p2_pool, psum_mm)
            consts = (w1f, w2f, w3, bh, cbase_f)
            aps = (x_query, x_support, x_cls, p2d_list, c3d_list, out0, out1)
            for n in range(per_core):
                _emit_image(nc, pools, consts, aps, n, stage=stage)

    nc.compile()
    return nc


_CACHED = {}


def _get_program(per_core=PER_CORE):
    key = (per_core, MM_DT)
    if key not in _CACHED:
        _CACHED[key] = build_program(per_core)
    return _CACHED[key]


def run(inputs, trace=False, per_core=PER_CORE):
    nc = _get_program(per_core)
    n_cores = N_FULL // per_core
    xq = np.ascontiguousarray(inputs["x_query"], dtype=np.float32)
    xs = np.ascontiguousarray(inputs["x_support"], dtype=np.float32)
    xc = np.ascontiguousarray(inputs["x_support_cls"], dtype=np.float32).reshape(
        N_FULL, S * D
    )
    wh = np.ascontiguousarray(inputs["W_head"], dtype=np.float32).reshape(3 * D, 1)
    bhv = np.ascontiguousarray(inputs["b_head"], dtype=np.float32).reshape(1, 1)
    in_maps = []
    for c in range(n_cores):
        sl = slice(c * per_core, (c + 1) * per_core)
        in_maps.append({
            "x_query": xq[sl], "x_support": xs[sl], "x_support_cls": xc[sl],
            "W_head": wh, "b_head": bhv,
        })
    res = run_bass_kernel_spmd(nc, in_maps, list(range(n_cores)), trace=trace)
    o0 = np.concatenate([res.results[c]["out0"] for c in range(n_cores)], axis=0)
    o1 = np.concatenate([res.results[c]["out1"] for c in range(n_cores)], axis=0)
    o0 = o0.reshape(N_FULL, 1, SIDE, SIDE).astype(np.float32)
    o1 = o1.reshape(N_FULL, 1, SIDE, SIDE).astype(np.float32)
    return (o0, o1), res


def kernel(**inputs):
    (o0, o1), _ = run(inputs, trace=False)
    return o0, o1


# revision 23
# speedup vs baseline: 1.1472x; 1.1472x over previous
"""Trainium2 Bass kernel: FADEv4 retrieval-kNN head (nn_FADEv4_7026566496861).

Math (per image n):
    cls  = l2norm(mean_s(x_support_cls[n]))          # [1,D]
    q    = l2norm(x_query[n])                        # [Tq,D]
    s    = l2norm(x_support[n])                      # [Ts,D]
    sim  = q @ s.T                                   # [Tq,Ts]
    dmin = 1 - max_ts(sim); idx = argmax_ts(sim)
    pred = sigmoid(q@W1 + s[idx]@W2 + cls@W3 + b)
    out0 = (pred*dmin).reshape(N,1,37,37); out1 = pred.reshape(N,1,37,37)

Sharding: data-parallel over N=16 images -> 8 cores x 2 images, no collectives.

v2 design (vs v1 which used PE transposes + bf16 matmuls):
  * Normalization pipeline per 128-token tile:
      ACT Square+accum (ssq) -> ACT Sqrt(scale=1/768) (denom = |x|*sqrt(1/768))
      -> GPSIMD normalize_recip (divide + cast to bf16; so normalized rows are
      sqrt(768)*x_hat) -> XBAR dma_start_transpose (bf16, [tok,768]->[128,6,tok])
      -> ACT copy-cast bf16->fp8e4.
    No PE transposes, no PSUM->SBUF copies.
  * Big matmul in fp8e4 with MatmulPerfMode.DoubleRow (2 contraction chunks
    per instruction, 2x PE throughput): sim_psum = 768 * cosine.
  * W1 appended as an extra support column (p1 = 768*q_hat@W1 falls out of the
    matmul); W2 as an extra query column (p2 row staged to DRAM, gathered per
    query row by indirect DMA).
  * Supports processed in NG=3 column groups (2048/2048/1408 incl W1) with
    [128,2048] f32 PSUM tiles (4 banks x 2 bufs); DVE max8 + max_index per
    (m-block, group).
  * Combine/argmax/head fully batched per image ([128, MB]-shaped ops).
"""

import math
import os

import numpy as np

import concourse.bass as bass
import concourse.mybir as mybir
import concourse.tile as tile
from concourse import bacc
from concourse.bass import ds, IndirectOffsetOnAxis
from concourse.bass_utils import run_bass_kernel_spmd

F32 = mybir.dt.float32
BF16 = mybir.dt.bfloat16
FP8 = mybir.dt.float8e4
U32 = mybir.dt.uint32
AX = mybir.AxisListType
OP = mybir.AluOpType
ACTF = mybir.ActivationFunctionType
PERF = mybir.MatmulPerfMode

N_FULL, TQ, TS, S, D = 16, 1369, 5476, 4, 768
SIDE = 37
KC = D // 128                       # 6 contraction chunks of 128
SSCALE = float(np.float32(1.0 / 768.0))   # sim_psum = cos / SSCALE
WSCALE = float(1.0 / math.sqrt(1.0 / 768.0))  # scale for W1/W2 columns
W2COL = 1376                        # W2 column in qT
TQE = W2COL + 1                     # 1377 qT columns
QT_PITCH = 1392                     # qTq k-chunk pitch; %16==0 for dual-fp8 LDW
MB = (TQE + 127) // 128             # 11 m-blocks (last: 97 cols, 89 real)
GW = 2048                           # support group width (4 PSUM banks)
NG = 3                              # groups: 2048, 2048, 1408
G2W = 1408                          # g2 sTq extent (512+512+384 MM chunks)
G2REAL = 1380                       # real supports in g2; W1 at col 1380
G2TB = 1392                         # extent written into sTb for g2
QTB_W = 1376                        # qTb extent (1280 + 96 padded edge)
P2D_LEN = TS                        # staged p2 covers exactly the real supports
NEG = -1.0e30
W_ALL = MB * NG * 8                 # 264 combine width

N_CORES = 8
PER_CORE = N_FULL // N_CORES

USE_DR = os.environ.get("FADE_DR", "1") == "1"   # DoubleRow fp8; else bf16
MM_DT = FP8 if USE_DR else BF16


def q_tiles():
    t = [(i * 128, 128, 128) for i in range(10)]
    t.append((1280, 89, 96))
    return t


def s_tiles(g):
    base = g * GW
    if g < 2:
        return [(base + i * 128, 128, 128) for i in range(16)]
    t = [(base + i * 128, 128, 128) for i in range(10)]
    t.append((base + 1280, 100, 112))
    return t


def _mm_chunks(g):
    if g < NG - 1:
        return [(0, 512), (512, 512), (1024, 512), (1536, 512)]
    return [(0, 512), (512, 512), (1024, 384)]


def _emit_image(nc, pools, consts, aps, n, stage=99):
    (img_pool, nmb_pool, stb_pool, stq_pool, scratch, cls_pool, p2_pool,
     psum_mm) = pools
    (w1f, w2f, w3, bh, cbase_f) = consts
    (x_query, x_support, x_cls, p2d_list, c3d_list, out0, out1) = aps

    # ---- cls head scalar: c3b = (cls_hat . W3) + b, broadcast to [128,1] ----
    clsbig = cls_pool.tile([1, S * D], F32, tag="clsbig")
    nc.sync.dma_start(out=clsbig[:, :], in_=x_cls[n])
    clsum = cls_pool.tile([1, D], F32, tag="clsum")
    nc.vector.tensor_add(clsum[:, :], clsbig[:, 0:D], clsbig[:, D:2 * D])
    nc.vector.tensor_add(clsum[:, :], clsum[:, :], clsbig[:, 2 * D:3 * D])
    nc.vector.tensor_add(clsum[:, :], clsum[:, :], clsbig[:, 3 * D:4 * D])
    sc3 = cls_pool.tile([1, D], F32, tag="sc3")
    ss3 = cls_pool.tile([1, 8], F32, tag="ss3")
    nc.vector.tensor_mul(sc3[:, :], clsum[:, :], clsum[:, :])
    nc.vector.tensor_reduce(out=ss3[:, 0:1], in_=sc3[:, :], axis=AX.X, op=OP.add)
    nc.vector.tensor_mul(sc3[:, :], clsum[:, :], w3[:, :])
    nc.vector.tensor_reduce(out=ss3[:, 1:2], in_=sc3[:, :], axis=AX.X, op=OP.add)
    nc.scalar.sqrt(ss3[:, 2:3], ss3[:, 0:1])
    nc.vector.reciprocal(ss3[:, 3:4], ss3[:, 2:3])
    nc.vector.tensor_mul(ss3[:, 4:5], ss3[:, 1:2], ss3[:, 3:4])
    nc.vector.tensor_add(ss3[:, 5:6], ss3[:, 4:5], bh[:, 0:1])
    nc.sync.dma_start(out=c3d_list[n][:, :], in_=ss3[0:1, 5:6])
    c3b = img_pool.tile([128, 1], F32, tag="c3b")
    nc.sync.dma_start(out=c3b[:, :], in_=c3d_list[n][:, :].to_broadcast((128, 1)))

    # ---- normalize a list of token tiles, then XBAR-transpose them ----
    # Phase-split so no hwdge queue head-blocks on the gpsimd divide: all
    # loads + norm passes first (norms alternate ACT/DVE), then all
    # transposes issued back-to-back on the Sync queue.
    def build_tiles(src_row0, tiles, dstb, base):
        # Per pair of tiles: one DMA load, two norm passes (DVE+ACT), one
        # sqrt, two gpsimd divides. raw lifetime stays one pair so the
        # scratch pool never over-commits. Transposes are issued at the end,
        # back-to-back on the Sync hwdge queue (nmb pool holds the group).
        nmbs = []

        use_ttr = os.environ.get("FADE_TTR", "0") == "1"

        def norm_pair(pairs):
            for j, (rawap, rp0) in enumerate(pairs):
                sden = scratch.tile([128, 1], F32, tag="sden")
                if rp0 < 128:
                    nc.vector.memset(sden[:, :], 1.0)
                if use_ttr and j == 0:
                    sqv = scratch.tile([128, D], BF16, tag="sqv")
                    nc.vector.tensor_tensor_reduce(
                        out=sqv[:rp0, :], in0=rawap[:rp0, :],
                        in1=rawap[:rp0, :], scale=1.0, scalar=0.0,
                        op0=OP.mult, op1=OP.add,
                        accum_out=sden[:rp0, :],
                    )
                else:
                    sqa = scratch.tile([128, D], BF16, tag="sqa")
                    nc.scalar.activation(
                        sqa[:rp0, :], rawap[:rp0, :], ACTF.Square,
                        accum_out=sden[:rp0, :],
                    )
                sdq = scratch.tile([128, 1], F32, tag="sdq")
                nc.scalar.activation(sdq[:, :], sden[:, :], ACTF.Sqrt,
                                     scale=SSCALE)
                nmb = nmb_pool.tile([128, D], BF16, tag="nmb")
                nc.gpsimd.normalize_recip(nmb[:rp0, :], rawap[:rp0, :],
                                          sdq[:rp0, :])
                tok0, _, trp = tiles[len(nmbs)]
                eng = nc.sync if (len(nmbs) % 2 == 0) else nc.scalar
                eng.dma_start_transpose(
                    out=dstb[:, :, ds(tok0 - base, trp)], in_=nmb[:trp, :]
                )
                nmbs.append(nmb)

        i = 0
        while i < len(tiles):
            t0, r0, rp0 = tiles[i]
            if i + 1 < len(tiles) and r0 == 128 and tiles[i + 1][1] == 128:
                raw2 = scratch.tile([128, 2, D], F32, tag="raw2")
                nc.sync.dma_start(
                    out=raw2[:, :, :],
                    in_=src_row0[ds(t0, 256), :].rearrange(
                        "(i p) d -> p i d", p=128),
                )
                norm_pair([(raw2[:, 0, :], 128), (raw2[:, 1, :], 128)])
                i += 2
            else:
                raw = scratch.tile([128, D], F32, tag="raw")
                if r0 < rp0:
                    nc.vector.memset(raw[:, :], 1.0)
                nc.sync.dma_start(out=raw[:r0, :], in_=src_row0[ds(t0, r0), :])
                norm_pair([(raw[:, :], rp0)])
                i += 1

    # ---- build qT (bf16 staged, then cast to fp8) ----
    qTb = stb_pool.tile([128, KC, GW], BF16, tag="sTb")
    build_tiles(x_query[n], q_tiles(), qTb, 0)
    qTq = img_pool.tile([128, KC, QT_PITCH], MM_DT, tag="qTq")
    for (c0, c1) in ((0, 768), (768, QTB_W)):
        nc.scalar.copy(qTq[:, :, c0:c1], qTb[:, :, c0:c1])
    nc.vector.memset(qTq[:, :, TQ:W2COL], 0)
    nc.vector.tensor_copy(
        qTq[:, :, W2COL:W2COL + 1].rearrange("p a b -> p (a b)"), w2f[:, :]
    )

    if stage < 2:
        z0 = scratch.tile([128, MB], F32, tag="z0")
        nc.vector.memset(z0[:, :], 0.25)
        for m in range(MB):
            mreal = 128 if m < MB - 1 else TQ - 128 * (MB - 1)
            nc.sync.dma_start(out=out1[n, ds(m * 128, mreal)], in_=z0[:mreal, m:m + 1])
            nc.sync.dma_start(out=out0[n, ds(m * 128, mreal)], in_=z0[:mreal, m:m + 1])
        return

    # ---- per-image state ----
    Mc8 = img_pool.tile([128, MB, NG, 8], F32, tag="Mc8")
    Ic8 = img_pool.tile([128, MB, NG, 8], U32, tag="Ic8")
    p1_all = img_pool.tile([128, MB], F32, tag="p1")
    # rows >= 97 of the last m-block are never written by max8; give them
    # defined values so the batched combine stays finite (clamp handles the
    # all-equal one-hot sum).
    nc.vector.memset(Mc8[:, :, :, :].rearrange("p a b c -> p (a b c)"), NEG)
    nc.vector.memset(Ic8[:, :, :, :].rearrange("p a b c -> p (a b c)"), 0)
    nc.vector.memset(p1_all[:, :], 0)

    def build_group(g):
        sTb = stb_pool.tile([128, KC, GW], BF16, tag="sTb")
        build_tiles(x_support[n], s_tiles(g), sTb, g * GW)
        sTq = stq_pool.tile([128, KC, GW], MM_DT, tag="sTq")
        if g < NG - 1:
            for (c0, c1) in ((0, 1024), (1024, 2048)):
                nc.scalar.copy(sTq[:, :, c0:c1], sTb[:, :, c0:c1])
        else:
            for (c0, c1) in ((0, 1024), (1024, G2TB)):
                nc.scalar.copy(sTq[:, :, c0:c1], sTb[:, :, c0:c1])
            nc.vector.memset(sTq[:, :, G2TB:G2W], 0)
            nc.vector.tensor_copy(
                sTq[:, :, G2REAL:G2REAL + 1].rearrange("p a b -> p (a b)"),
                w1f[:, :],
            )
        return sTq

    def mm_group(g, sTq):
        gn = GW if g < NG - 1 else G2REAL
        for m in range(MB):
            mcols = 128 if m < MB - 1 else TQE - 128 * (MB - 1)
            bp = psum_mm.tile([128, GW], F32, tag="bp")
            if USE_DR:
                for (c0, cw) in _mm_chunks(g):
                    for k2 in range(KC // 2):
                        nc.tensor.matmul(
                            bp[:mcols, ds(c0, cw)],
                            lhsT=qTq[:, 2 * k2:2 * k2 + 2, ds(m * 128, mcols)],
                            rhs=sTq[:, 2 * k2:2 * k2 + 2, ds(c0, cw)],
                            start=(k2 == 0), stop=(k2 == KC // 2 - 1),
                            perf_mode=PERF.DoubleRow,
                        )
            else:
                for (c0, cw) in _mm_chunks(g):
                    for k in range(KC):
                        nc.tensor.matmul(
                            bp[:mcols, ds(c0, cw)],
                            lhsT=qTq[:, k, ds(m * 128, mcols)],
                            rhs=sTq[:, k, ds(c0, cw)],
                            start=(k == 0), stop=(k == KC - 1),
                        )
            nc.vector.max(Mc8[:mcols, m, g, :], bp[:mcols, :gn])
            nc.vector.max_index(
                Ic8[:mcols, m, g, :], Mc8[:mcols, m, g, :], bp[:mcols, :gn]
            )
            if g == NG - 1:
                nc.scalar.copy(p1_all[:mcols, m:m + 1],
                               bp[:mcols, G2REAL:G2REAL + 1])
            if m == MB - 1:
                p2row = p2_pool.tile([1, GW], F32, tag="p2row")
                nc.scalar.copy(p2row[0:1, :gn], bp[96:97, :gn])
                nc.sync.dma_start(
                    out=p2d_list[n][ds(g * GW, gn), 0], in_=p2row[0:1, :gn]
                )

    # software-pipelined: build group g+1 before the matmuls of group g
    sT0 = build_group(0)
    sT1 = build_group(1)
    mm_group(0, sT0)
    sT2 = build_group(2)
    mm_group(1, sT1)
    mm_group(2, sT2)

    if stage < 3:
        z0 = scratch.tile([128, MB], F32, tag="z0")
        nc.vector.tensor_copy(z0[:, :], Mc8[:, :, 0, 0])
        for m in range(MB):
            mreal = 128 if m < MB - 1 else TQ - 128 * (MB - 1)
            nc.sync.dma_start(out=out1[n, ds(m * 128, mreal)], in_=z0[:mreal, m:m + 1])
            nc.sync.dma_start(out=out0[n, ds(m * 128, mreal)], in_=p1_all[:mreal, m:m + 1])
        return

    # ---- batched combine: global max / argmax per m-block ----
    mcA = Mc8[:, :, :, :].rearrange("p a b c -> p a (b c)")   # [128, MB, 24]
    icA = Ic8[:, :, :, :].rearrange("p a b c -> p (a b c)")   # [128, 264]
    gmax = img_pool.tile([128, MB], F32, tag="gmax")
    nc.vector.tensor_reduce(out=gmax[:, :], in_=mcA, axis=AX.X, op=OP.max)
    oneh = img_pool.tile([128, MB, NG * 8], F32, tag="oneh")
    for m in range(MB):
        nc.vector.tensor_scalar(
            out=oneh[:, m, :], in0=mcA[:, m, :], scalar1=gmax[:, m:m + 1],
            scalar2=None, op0=OP.is_equal,
        )
    icf = img_pool.tile([128, MB, NG * 8], F32, tag="icf")
    icfF = icf[:, :, :].rearrange("p a b -> p (a b)")
    nc.vector.tensor_copy(icfF, icA)
    nc.vector.tensor_add(icfF, icfF, cbase_f[:, :])
    nc.vector.tensor_mul(icfF, icfF,
                         oneh[:, :, :].rearrange("p a b -> p (a b)"))
    gidxf = img_pool.tile([128, MB], F32, tag="gidxf")
    nc.vector.tensor_reduce(
        out=gidxf[:, :], in_=icf[:, :, :], axis=AX.X, op=OP.add,
    )
    nc.vector.tensor_scalar_min(gidxf[:, :], gidxf[:, :], float(TS - 1))
    gidx = img_pool.tile([128, MB], U32, tag="gidx")
    nc.vector.tensor_copy(gidx[:, :], gidxf[:, :])
    dmin_all = img_pool.tile([128, MB], F32, tag="dmin")
    nc.scalar.activation(dmin_all[:, :], gmax[:, :], ACTF.Copy,
                         bias=1.0, scale=-SSCALE)

    # ---- p2 gather + batched head ----
    p2g = img_pool.tile([128, MB], F32, tag="p2g")
    for m in range(MB):
        nc.gpsimd.indirect_dma_start(
            out=p2g[:, m:m + 1], out_offset=None, in_=p2d_list[n][:, :],
            in_offset=IndirectOffsetOnAxis(ap=gidx[:, m:m + 1], axis=0),
        )
    lg = img_pool.tile([128, MB], F32, tag="lg")
    nc.vector.tensor_add(lg[:, :], p1_all[:, :], p2g[:, :])
    pred = img_pool.tile([128, MB], F32, tag="pred")
    nc.scalar.activation(pred[:, :], lg[:, :], ACTF.Sigmoid,
                         bias=c3b[:, 0:1], scale=SSCALE)
    o0 = img_pool.tile([128, MB], F32, tag="o0")
    nc.vector.tensor_mul(o0[:, :], pred[:, :], dmin_all[:, :])
    for m in range(MB):
        mreal = 128 if m < MB - 1 else TQ - 128 * (MB - 1)
        nc.sync.dma_start(out=out1[n, ds(m * 128, mreal)], in_=pred[:mreal, m:m + 1])
        nc.sync.dma_start(out=out0[n, ds(m * 128, mreal)], in_=o0[:mreal, m:m + 1])


def build_program(per_core=PER_CORE, stage=99):
    nc = bacc.Bacc("TRN2", target_bir_lowering=False, debug=False)
    x_query = nc.dram_tensor("x_query", [per_core, TQ, D], F32, kind="ExternalInput").ap()
    x_support = nc.dram_tensor("x_support", [per_core, TS, D], F32, kind="ExternalInput").ap()
    x_cls = nc.dram_tensor("x_support_cls", [per_core, S * D], F32, kind="ExternalInput").ap()
    w_head = nc.dram_tensor("W_head", [3 * D, 1], F32, kind="ExternalInput").ap()
    b_head = nc.dram_tensor("b_head", [1, 1], F32, kind="ExternalInput").ap()
    out0 = nc.dram_tensor("out0", [per_core, TQ], F32, kind="ExternalOutput").ap()
    out1 = nc.dram_tensor("out1", [per_core, TQ], F32, kind="ExternalOutput").ap()
    p2d_list = [
        nc.dram_tensor(f"p2d_{n}", [P2D_LEN, 1], F32).ap() for n in range(per_core)
    ]
    c3d_list = [
        nc.dram_tensor(f"c3d_{n}", [1, 1], F32).ap() for n in range(per_core)
    ]

    with tile.TileContext(nc) as tc:
        with tc.tile_pool(name="img", bufs=2) as img_pool, \
             tc.tile_pool(name="nmbp", bufs=18) as nmb_pool, \
             tc.tile_pool(name="stb", bufs=2) as stb_pool, \
             tc.tile_pool(name="stq", bufs=2) as stq_pool, \
             tc.tile_pool(name="scratch", bufs=4) as scratch, \
             tc.tile_pool(name="cls", bufs=1) as cls_pool, \
             tc.tile_pool(name="p2", bufs=1) as p2_pool, \
             tc.tile_pool(name="const", bufs=1) as const_pool, \
             tc.tile_pool(name="psum_mm", bufs=2, space="PSUM") as psum_mm:

            # constants
            w1s = const_pool.tile([128, KC], F32)
            w2s = const_pool.tile([128, KC], F32)
            w3 = const_pool.tile([1, D], F32)
            bh = const_pool.tile([1, 1], F32)
            for k in range(KC):
                nc.sync.dma_start(out=w1s[:, k:k + 1], in_=w_head[ds(128 * k, 128), :])
                nc.sync.dma_start(out=w2s[:, k:k + 1], in_=w_head[ds(D + 128 * k, 128), :])
            nc.sync.dma_start(out=w3[0:1, :], in_=w_head[ds(2 * D, D), :])
            nc.sync.dma_start(out=bh[:, :], in_=b_head[:, :])
            w1f = const_pool.tile([128, KC], MM_DT)
            w2f = const_pool.tile([128, KC], MM_DT)
            nc.scalar.activation(w1f[:, :], w1s[:, :], ACTF.Copy, scale=WSCALE)
            nc.scalar.activation(w2f[:, :], w2s[:, :], ACTF.Copy, scale=WSCALE)
            cbase_u = const_pool.tile([128, W_ALL], U32)
            cbase_f = const_pool.tile([128, W_ALL], F32)
            nc.gpsimd.iota(cbase_u[:, :], pattern=[[0, MB], [GW, NG], [0, 8]],
                           base=0, channel_multiplier=0)
            nc.vector.tensor_copy(cbase_f[:, :], cbase_u[:, :])

            pools = (img_pool, nmb_pool, stb_pool, stq_pool, scratch, cls_pool,
                     p2_pool, psum_mm)
            consts = (w1f, w2f, w3, bh, cbase_f)
            aps = (x_query, x_support, x_cls, p2d_list, c3d_list, out0, out1)
            for n in range(per_core):
                _emit_image(nc, pools, consts, aps, n, stage=stage)

    nc.compile()
    return nc


_CACHED = {}


def _get_program(per_core=PER_CORE):
    key = (per_core, MM_DT)
    if key not in _CACHED:
        _CACHED[key] = build_program(per_core)
    return _CACHED[key]


def run(inputs, trace=False, per_core=PER_CORE):
    nc = _get_program(per_core)
    n_cores = N_FULL // per_core
    xq = np.ascontiguousarray(inputs["x_query"], dtype=np.float32)
    xs = np.ascontiguousarray(inputs["x_support"], dtype=np.float32)
    xc = np.ascontiguousarray(inputs["x_support_cls"], dtype=np.float32).reshape(
        N_FULL, S * D
    )
    wh = np.ascontiguousarray(inputs["W_head"], dtype=np.float32).reshape(3 * D, 1)
    bhv = np.ascontiguousarray(inputs["b_head"], dtype=np.float32).reshape(1, 1)
    in_maps = []
    for c in range(n_cores):
        sl = slice(c * per_core, (c + 1) * per_core)
        in_maps.append({
            "x_query": xq[sl], "x_support": xs[sl], "x_support_cls": xc[sl],
            "W_head": wh, "b_head": bhv,
        })
    res = run_bass_kernel_spmd(nc, in_maps, list(range(n_cores)), trace=trace)
    o0 = np.concatenate([res.results[c]["out0"] for c in range(n_cores)], axis=0)
    o1 = np.concatenate([res.results[c]["out1"] for c in range(n_cores)], axis=0)
    o0 = o0.reshape(N_FULL, 1, SIDE, SIDE).astype(np.float32)
    o1 = o1.reshape(N_FULL, 1, SIDE, SIDE).astype(np.float32)
    return (o0, o1), res


def kernel(**inputs):
    (o0, o1), _ = run(inputs, trace=False)
    return o0, o1


# revision 25
# speedup vs baseline: 1.1521x; 1.0043x over previous
"""Trainium2 Bass kernel: FADEv4 retrieval-kNN head (nn_FADEv4_7026566496861).

Math (per image n):
    cls  = l2norm(mean_s(x_support_cls[n]))          # [1,D]
    q    = l2norm(x_query[n])                        # [Tq,D]
    s    = l2norm(x_support[n])                      # [Ts,D]
    sim  = q @ s.T                                   # [Tq,Ts]
    dmin = 1 - max_ts(sim); idx = argmax_ts(sim)
    pred = sigmoid(q@W1 + s[idx]@W2 + cls@W3 + b)
    out0 = (pred*dmin).reshape(N,1,37,37); out1 = pred.reshape(N,1,37,37)

Sharding: data-parallel over N=16 images -> 8 cores x 2 images, no collectives.

v2 design (vs v1 which used PE transposes + bf16 matmuls):
  * Normalization pipeline per 128-token tile:
      ACT Square+accum (ssq) -> ACT Sqrt(scale=1/768) (denom = |x|*sqrt(1/768))
      -> GPSIMD normalize_recip (divide + cast to bf16; so normalized rows are
      sqrt(768)*x_hat) -> XBAR dma_start_transpose (bf16, [tok,768]->[128,6,tok])
      -> ACT copy-cast bf16->fp8e4.
    No PE transposes, no PSUM->SBUF copies.
  * Big matmul in fp8e4 with MatmulPerfMode.DoubleRow (2 contraction chunks
    per instruction, 2x PE throughput): sim_psum = 768 * cosine.
  * W1 appended as an extra support column (p1 = 768*q_hat@W1 falls out of the
    matmul); W2 as an extra query column (p2 row staged to DRAM, gathered per
    query row by indirect DMA).
  * Supports processed in NG=3 column groups (2048/2048/1408 incl W1) with
    [128,2048] f32 PSUM tiles (4 banks x 2 bufs); DVE max8 + max_index per
    (m-block, group).
  * Combine/argmax/head fully batched per image ([128, MB]-shaped ops).
"""

import math
import os

import numpy as np

import concourse.bass as bass
import concourse.mybir as mybir
import concourse.tile as tile
from concourse import bacc
from concourse.bass import ds, IndirectOffsetOnAxis
from concourse.bass_utils import run_bass_kernel_spmd

F32 = mybir.dt.float32
BF16 = mybir.dt.bfloat16
FP8 = mybir.dt.float8e4
U32 = mybir.dt.uint32
AX = mybir.AxisListType
OP = mybir.AluOpType
ACTF = mybir.ActivationFunctionType
PERF = mybir.MatmulPerfMode

N_FULL, TQ, TS, S, D = 16, 1369, 5476, 4, 768
SIDE = 37
KC = D // 128                       # 6 contraction chunks of 128
SSCALE = float(np.float32(1.0 / 768.0))   # sim_psum = cos / SSCALE
WSCALE = float(1.0 / math.sqrt(1.0 / 768.0))  # scale for W1/W2 columns
W2COL = 1376                        # W2 column in qT
TQE = W2COL + 1                     # 1377 qT columns
QT_PITCH = 1392                     # qTq k-chunk pitch; %16==0 for dual-fp8 LDW
MB = (TQE + 127) // 128             # 11 m-blocks (last: 97 cols, 89 real)
GW = 2048                           # support group width (4 PSUM banks)
NG = 3                              # groups: 2048, 2048, 1408
G2W = 1408                          # g2 sTq extent (512+512+384 MM chunks)
G2REAL = 1380                       # real supports in g2; W1 at col 1380
G2TB = 1392                         # extent written into sTb for g2
QTB_W = 1376                        # qTb extent (1280 + 96 padded edge)
P2D_LEN = TS                        # staged p2 covers exactly the real supports
NEG = -1.0e30
W_ALL = MB * NG * 8                 # 264 combine width

N_CORES = 8
PER_CORE = N_FULL // N_CORES

USE_DR = os.environ.get("FADE_DR", "1") == "1"   # DoubleRow fp8; else bf16
MM_DT = FP8 if USE_DR else BF16


def q_tiles():
    t = [(i * 128, 128, 128) for i in range(10)]
    t.append((1280, 89, 96))
    return t


def s_tiles(g):
    base = g * GW
    if g < 2:
        return [(base + i * 128, 128, 128) for i in range(16)]
    t = [(base + i * 128, 128, 128) for i in range(10)]
    t.append((base + 1280, 100, 112))
    return t


def _mm_chunks(g):
    if g < NG - 1:
        return [(0, 512), (512, 512), (1024, 512), (1536, 512)]
    return [(0, 512), (512, 512), (1024, 384)]


def _emit_image(nc, pools, consts, aps, n, stage=99):
    (img_pool, nmb_pool, stb_pool, stq_pool, scratch, cls_pool, p2_pool,
     psum_mm) = pools
    (w1f, w2f, w3, bh, cbase_f) = consts
    (x_query, x_support, x_cls, p2d_list, c3d_list, out0, out1) = aps

    # ---- cls head scalar: c3b = (cls_hat . W3) + b, broadcast to [128,1] ----
    clsbig = cls_pool.tile([1, S * D], F32, tag="clsbig")
    nc.sync.dma_start(out=clsbig[:, :], in_=x_cls[n])
    clsum = cls_pool.tile([1, D], F32, tag="clsum")
    nc.vector.tensor_add(clsum[:, :], clsbig[:, 0:D], clsbig[:, D:2 * D])
    nc.vector.tensor_add(clsum[:, :], clsum[:, :], clsbig[:, 2 * D:3 * D])
    nc.vector.tensor_add(clsum[:, :], clsum[:, :], clsbig[:, 3 * D:4 * D])
    sc3 = cls_pool.tile([1, D], F32, tag="sc3")
    ss3 = cls_pool.tile([1, 8], F32, tag="ss3")
    nc.vector.tensor_mul(sc3[:, :], clsum[:, :], clsum[:, :])
    nc.vector.tensor_reduce(out=ss3[:, 0:1], in_=sc3[:, :], axis=AX.X, op=OP.add)
    nc.vector.tensor_mul(sc3[:, :], clsum[:, :], w3[:, :])
    nc.vector.tensor_reduce(out=ss3[:, 1:2], in_=sc3[:, :], axis=AX.X, op=OP.add)
    nc.scalar.sqrt(ss3[:, 2:3], ss3[:, 0:1])
    nc.vector.reciprocal(ss3[:, 3:4], ss3[:, 2:3])
    nc.vector.tensor_mul(ss3[:, 4:5], ss3[:, 1:2], ss3[:, 3:4])
    nc.vector.tensor_add(ss3[:, 5:6], ss3[:, 4:5], bh[:, 0:1])
    nc.sync.dma_start(out=c3d_list[n][:, :], in_=ss3[0:1, 5:6])
    c3b = img_pool.tile([128, 1], F32, tag="c3b")
    nc.sync.dma_start(out=c3b[:, :], in_=c3d_list[n][:, :].to_broadcast((128, 1)))

    # ---- normalize a list of token tiles, then XBAR-transpose them ----
    # Phase-split so no hwdge queue head-blocks on the gpsimd divide: all
    # loads + norm passes first (norms alternate ACT/DVE), then all
    # transposes issued back-to-back on the Sync queue.
    def build_tiles(src_row0, tiles, dstb, base):
        # Per pair of tiles: one DMA load, two norm passes (DVE+ACT), one
        # sqrt, two gpsimd divides. raw lifetime stays one pair so the
        # scratch pool never over-commits. Transposes are issued at the end,
        # back-to-back on the Sync hwdge queue (nmb pool holds the group).
        nmbs = []

        use_ttr = os.environ.get("FADE_TTR", "0") == "1"

        def norm_pair(pairs):
            for j, (rawap, rp0) in enumerate(pairs):
                sden = scratch.tile([128, 1], F32, tag="sden")
                if rp0 < 128:
                    nc.vector.memset(sden[:, :], 1.0)
                if use_ttr and j == 0:
                    sqv = scratch.tile([128, D], BF16, tag="sqv")
                    nc.vector.tensor_tensor_reduce(
                        out=sqv[:rp0, :], in0=rawap[:rp0, :],
                        in1=rawap[:rp0, :], scale=1.0, scalar=0.0,
                        op0=OP.mult, op1=OP.add,
                        accum_out=sden[:rp0, :],
                    )
                else:
                    sqa = scratch.tile([128, D], BF16, tag="sqa")
                    nc.scalar.activation(
                        sqa[:rp0, :], rawap[:rp0, :], ACTF.Square,
                        accum_out=sden[:rp0, :],
                    )
                sdq = scratch.tile([128, 1], F32, tag="sdq")
                nc.scalar.activation(sdq[:, :], sden[:, :], ACTF.Sqrt,
                                     scale=SSCALE)
                nmb = nmb_pool.tile([128, D], BF16, tag="nmb")
                nc.gpsimd.normalize_recip(nmb[:rp0, :], rawap[:rp0, :],
                                          sdq[:rp0, :])
                nmbs.append(nmb)

        # Emit transposes one pair BEHIND the norm chain so the hwdge queue
        # never head-blocks waiting on an in-flight gpsimd divide.
        tp_done = [0]

        def issue_tps(upto):
            while tp_done[0] < upto:
                j = tp_done[0]
                tok0, _, trp = tiles[j]
                eng = nc.sync if (j % 2 == 0) else nc.scalar
                eng.dma_start_transpose(
                    out=dstb[:, :, ds(tok0 - base, trp)], in_=nmbs[j][:trp, :]
                )
                tp_done[0] += 1

        i = 0
        while i < len(tiles):
            t0, r0, rp0 = tiles[i]
            if i + 1 < len(tiles) and r0 == 128 and tiles[i + 1][1] == 128:
                raw2 = scratch.tile([128, 2, D], F32, tag="raw2")
                nc.sync.dma_start(
                    out=raw2[:, :, :],
                    in_=src_row0[ds(t0, 256), :].rearrange(
                        "(i p) d -> p i d", p=128),
                )
                norm_pair([(raw2[:, 0, :], 128), (raw2[:, 1, :], 128)])
                i += 2
            else:
                raw = scratch.tile([128, D], F32, tag="raw")
                if r0 < rp0:
                    nc.vector.memset(raw[:, :], 1.0)
                nc.sync.dma_start(out=raw[:r0, :], in_=src_row0[ds(t0, r0), :])
                norm_pair([(raw[:, :], rp0)])
                i += 1
            issue_tps(len(nmbs) - 2)
        issue_tps(len(tiles))

    # ---- build qT (bf16 staged, then cast to fp8) ----
    qTb = stb_pool.tile([128, KC, GW], BF16, tag="sTb")
    build_tiles(x_query[n], q_tiles(), qTb, 0)
    qTq = img_pool.tile([128, KC, QT_PITCH], MM_DT, tag="qTq")
    for (c0, c1) in ((0, 768), (768, QTB_W)):
        nc.scalar.copy(qTq[:, :, c0:c1], qTb[:, :, c0:c1])
    nc.vector.memset(qTq[:, :, TQ:W2COL], 0)
    nc.vector.tensor_copy(
        qTq[:, :, W2COL:W2COL + 1].rearrange("p a b -> p (a b)"), w2f[:, :]
    )

    if stage < 2:
        z0 = scratch.tile([128, MB], F32, tag="z0")
        nc.vector.memset(z0[:, :], 0.25)
        for m in range(MB):
            mreal = 128 if m < MB - 1 else TQ - 128 * (MB - 1)
            nc.sync.dma_start(out=out1[n, ds(m * 128, mreal)], in_=z0[:mreal, m:m + 1])
            nc.sync.dma_start(out=out0[n, ds(m * 128, mreal)], in_=z0[:mreal, m:m + 1])
        return

    # ---- per-image state ----
    Mc8 = img_pool.tile([128, MB, NG, 8], F32, tag="Mc8")
    Ic8 = img_pool.tile([128, MB, NG, 8], U32, tag="Ic8")
    p1_all = img_pool.tile([128, MB], F32, tag="p1")
    # rows >= 97 of the last m-block are never written by max8; give them
    # defined values so the batched combine stays finite (clamp handles the
    # all-equal one-hot sum).
    nc.vector.memset(Mc8[:, :, :, :].rearrange("p a b c -> p (a b c)"), NEG)
    nc.vector.memset(Ic8[:, :, :, :].rearrange("p a b c -> p (a b c)"), 0)
    nc.vector.memset(p1_all[:, :], 0)

    def build_group(g):
        sTb = stb_pool.tile([128, KC, GW], BF16, tag="sTb")
        build_tiles(x_support[n], s_tiles(g), sTb, g * GW)
        sTq = stq_pool.tile([128, KC, GW], MM_DT, tag="sTq")
        if g < NG - 1:
            for (c0, c1) in ((0, 1024), (1024, 2048)):
                nc.scalar.copy(sTq[:, :, c0:c1], sTb[:, :, c0:c1])
        else:
            for (c0, c1) in ((0, 1024), (1024, G2TB)):
                nc.scalar.copy(sTq[:, :, c0:c1], sTb[:, :, c0:c1])
            nc.vector.memset(sTq[:, :, G2TB:G2W], 0)
            nc.vector.tensor_copy(
                sTq[:, :, G2REAL:G2REAL + 1].rearrange("p a b -> p (a b)"),
                w1f[:, :],
            )
        return sTq

    def mm_group(g, sTq):
        gn = GW if g < NG - 1 else G2REAL
        for m in range(MB):
            mcols = 128 if m < MB - 1 else TQE - 128 * (MB - 1)
            bp = psum_mm.tile([128, GW], F32, tag="bp")
            if USE_DR:
                for (c0, cw) in _mm_chunks(g):
                    for k2 in range(KC // 2):
                        nc.tensor.matmul(
                            bp[:mcols, ds(c0, cw)],
                            lhsT=qTq[:, 2 * k2:2 * k2 + 2, ds(m * 128, mcols)],
                            rhs=sTq[:, 2 * k2:2 * k2 + 2, ds(c0, cw)],
                            start=(k2 == 0), stop=(k2 == KC // 2 - 1),
                            perf_mode=PERF.DoubleRow,
                        )
            else:
                for (c0, cw) in _mm_chunks(g):
                    for k in range(KC):
                        nc.tensor.matmul(
                            bp[:mcols, ds(c0, cw)],
                            lhsT=qTq[:, k, ds(m * 128, mcols)],
                            rhs=sTq[:, k, ds(c0, cw)],
                            start=(k == 0), stop=(k == KC - 1),
                        )
            nc.vector.max(Mc8[:mcols, m, g, :], bp[:mcols, :gn])
            nc.vector.max_index(
                Ic8[:mcols, m, g, :], Mc8[:mcols, m, g, :], bp[:mcols, :gn]
            )
            if g == NG - 1:
                nc.scalar.copy(p1_all[:mcols, m:m + 1],
                               bp[:mcols, G2REAL:G2REAL + 1])
            if m == MB - 1:
                p2row = p2_pool.tile([1, GW], F32, tag="p2row")
                nc.scalar.copy(p2row[0:1, :gn], bp[96:97, :gn])
                nc.sync.dma_start(
                    out=p2d_list[n][ds(g * GW, gn), 0], in_=p2row[0:1, :gn]
                )

    # software-pipelined: build group g+1 before the matmuls of group g
    sT0 = build_group(0)
    sT1 = build_group(1)
    mm_group(0, sT0)
    sT2 = build_group(2)
    mm_group(1, sT1)
    mm_group(2, sT2)

    if stage < 3:
        z0 = scratch.tile([128, MB], F32, tag="z0")
        nc.vector.tensor_copy(z0[:, :], Mc8[:, :, 0, 0])
        for m in range(MB):
            mreal = 128 if m < MB - 1 else TQ - 128 * (MB - 1)
            nc.sync.dma_start(out=out1[n, ds(m * 128, mreal)], in_=z0[:mreal, m:m + 1])
            nc.sync.dma_start(out=out0[n, ds(m * 128, mreal)], in_=p1_all[:mreal, m:m + 1])
        return

    # ---- batched combine: global max / argmax per m-block ----
    mcA = Mc8[:, :, :, :].rearrange("p a b c -> p a (b c)")   # [128, MB, 24]
    icA = Ic8[:, :, :, :].rearrange("p a b c -> p (a b c)")   # [128, 264]
    gmax = img_pool.tile([128, MB], F32, tag="gmax")
    nc.vector.tensor_reduce(out=gmax[:, :], in_=mcA, axis=AX.X, op=OP.max)
    oneh = img_pool.tile([128, MB, NG * 8], F32, tag="oneh")
    for m in range(MB):
        nc.vector.tensor_scalar(
            out=oneh[:, m, :], in0=mcA[:, m, :], scalar1=gmax[:, m:m + 1],
            scalar2=None, op0=OP.is_equal,
        )
    icf = img_pool.tile([128, MB, NG * 8], F32, tag="icf")
    icfF = icf[:, :, :].rearrange("p a b -> p (a b)")
    nc.vector.tensor_copy(icfF, icA)
    nc.vector.tensor_add(icfF, icfF, cbase_f[:, :])
    nc.vector.tensor_mul(icfF, icfF,
                         oneh[:, :, :].rearrange("p a b -> p (a b)"))
    gidxf = img_pool.tile([128, MB], F32, tag="gidxf")
    nc.vector.tensor_reduce(
        out=gidxf[:, :], in_=icf[:, :, :], axis=AX.X, op=OP.add,
    )
    nc.vector.tensor_scalar_min(gidxf[:, :], gidxf[:, :], float(TS - 1))
    gidx = img_pool.tile([128, MB], U32, tag="gidx")
    nc.vector.tensor_copy(gidx[:, :], gidxf[:, :])
    dmin_all = img_pool.tile([128, MB], F32, tag="dmin")
    nc.scalar.activation(dmin_all[:, :], gmax[:, :], ACTF.Copy,
                         bias=1.0, scale=-SSCALE)

    # ---- p2 gather + batched head ----
    p2g = img_pool.tile([128, MB], F32, tag="p2g")
    for m in range(MB):
        nc.gpsimd.indirect_dma_start(
            out=p2g[:, m:m + 1], out_offset=None, in_=p2d_list[n][:, :],
            in_offset=IndirectOffsetOnAxis(ap=gidx[:, m:m + 1], axis=0),
        )
    lg = img_pool.tile([128, MB], F32, tag="lg")
    nc.vector.tensor_add(lg[:, :], p1_all[:, :], p2g[:, :])
    pred = img_pool.tile([128, MB], F32, tag="pred")
    nc.scalar.activation(pred[:, :], lg[:, :], ACTF.Sigmoid,
                         bias=c3b[:, 0:1], scale=SSCALE)
    o0 = img_pool.tile([128, MB], F32, tag="o0")
    nc.vector.tensor_mul(o0[:, :], pred[:, :], dmin_all[:, :])
    for m in range(MB):
        mreal = 128 if m < MB - 1 else TQ - 128 * (MB - 1)
        nc.sync.dma_start(out=out1[n, ds(m * 128, mreal)], in_=pred[:mreal, m:m + 1])
        nc.sync.dma_start(out=out0[n, ds(m * 128, mreal)], in_=o0[:mreal, m:m + 1])


def build_program(per_core=PER_CORE, stage=99):
    nc = bacc.Bacc("TRN2", target_bir_lowering=False, debug=False)
    x_query = nc.dram_tensor("x_query", [per_core, TQ, D], F32, kind="ExternalInput").ap()
    x_support = nc.dram_tensor("x_support", [per_core, TS, D], F32, kind="ExternalInput").ap()
    x_cls = nc.dram_tensor("x_support_cls", [per_core, S * D], F32, kind="ExternalInput").ap()
    w_head = nc.dram_tensor("W_head", [3 * D, 1], F32, kind="ExternalInput").ap()
    b_head = nc.dram_tensor("b_head", [1, 1], F32, kind="ExternalInput").ap()
    out0 = nc.dram_tensor("out0", [per_core, TQ], F32, kind="ExternalOutput").ap()
    out1 = nc.dram_tensor("out1", [per_core, TQ], F32, kind="ExternalOutput").ap()
    p2d_list = [
        nc.dram_tensor(f"p2d_{n}", [P2D_LEN, 1], F32).ap() for n in range(per_core)
    ]
    c3d_list = [
        nc.dram_tensor(f"c3d_{n}", [1, 1], F32).ap() for n in range(per_core)
    ]

    with tile.TileContext(nc) as tc:
        with tc.tile_pool(name="img", bufs=2) as img_pool, \
             tc.tile_pool(name="nmbp", bufs=18) as nmb_pool, \
             tc.tile_pool(name="stb", bufs=2) as stb_pool, \
             tc.tile_pool(name="stq", bufs=2) as stq_pool, \
             tc.tile_pool(name="scratch", bufs=4) as scratch, \
             tc.tile_pool(name="cls", bufs=1) as cls_pool, \
             tc.tile_pool(name="p2", bufs=1) as p2_pool, \
             tc.tile_pool(name="const", bufs=1) as const_pool, \
             tc.tile_pool(name="psum_mm", bufs=2, space="PSUM") as psum_mm:

            # constants
            w1s = const_pool.tile([128, KC], F32)
            w2s = const_pool.tile([128, KC], F32)
            w3 = const_pool.tile([1, D], F32)
            bh = const_pool.tile([1, 1], F32)
            for k in range(KC):
                nc.sync.dma_start(out=w1s[:, k:k + 1], in_=w_head[ds(128 * k, 128), :])
                nc.sync.dma_start(out=w2s[:, k:k + 1], in_=w_head[ds(D + 128 * k, 128), :])
            nc.sync.dma_start(out=w3[0:1, :], in_=w_head[ds(2 * D, D), :])
            nc.sync.dma_start(out=bh[:, :], in_=b_head[:, :])
            w1f = const_pool.tile([128, KC], MM_DT)
            w2f = const_pool.tile([128, KC], MM_DT)
            nc.scalar.activation(w1f[:, :], w1s[:, :], ACTF.Copy, scale=WSCALE)
            nc.scalar.activation(w2f[:, :], w2s[:, :], ACTF.Copy, scale=WSCALE)
            cbase_u = const_pool.tile([128, W_ALL], U32)
            cbase_f = const_pool.tile([128, W_ALL], F32)
            nc.gpsimd.iota(cbase_u[:, :], pattern=[[0, MB], [GW, NG], [0, 8]],
                           base=0, channel_multiplier=0)
            nc.vector.tensor_copy(cbase_f[:, :], cbase_u[:, :])

            pools = (img_pool, nmb_pool, stb_pool, stq_pool, scratch, cls_pool,
                     p2_pool, psum_mm)
            consts = (w1f, w2f, w3, bh, cbase_f)
            aps = (x_query, x_support, x_cls, p2d_list, c3d_list, out0, out1)
            for n in range(per_core):
                _emit_image(nc, pools, consts, aps, n, stage=stage)

    nc.compile()
    return nc


_CACHED = {}


def _get_program(per_core=PER_CORE):
    key = (per_core, MM_DT)
    if key not in _CACHED:
        _CACHED[key] = build_program(per_core)
    return _CACHED[key]


def run(inputs, trace=False, per_core=PER_CORE):
    nc = _get_program(per_core)
    n_cores = N_FULL // per_core
    xq = np.ascontiguousarray(inputs["x_query"], dtype=np.float32)
    xs = np.ascontiguousarray(inputs["x_support"], dtype=np.float32)
    xc = np.ascontiguousarray(inputs["x_support_cls"], dtype=np.float32).reshape(
        N_FULL, S * D
    )
    wh = np.ascontiguousarray(inputs["W_head"], dtype=np.float32).reshape(3 * D, 1)
    bhv = np.ascontiguousarray(inputs["b_head"], dtype=np.float32).reshape(1, 1)
    in_maps = []
    for c in range(n_cores):
        sl = slice(c * per_core, (c + 1) * per_core)
        in_maps.append({
            "x_query": xq[sl], "x_support": xs[sl], "x_support_cls": xc[sl],
            "W_head": wh, "b_head": bhv,
        })
    res = run_bass_kernel_spmd(nc, in_maps, list(range(n_cores)), trace=trace)
    o0 = np.concatenate([res.results[c]["out0"] for c in range(n_cores)], axis=0)
    o1 = np.concatenate([res.results[c]["out1"] for c in range(n_cores)], axis=0)
    o0 = o0.reshape(N_FULL, 1, SIDE, SIDE).astype(np.float32)
    o1 = o1.reshape(N_FULL, 1, SIDE, SIDE).astype(np.float32)
    return (o0, o1), res


def kernel(**inputs):
    (o0, o1), _ = run(inputs, trace=False)
    return o0, o1


# revision 26
# speedup vs baseline: 1.1570x; 1.0042x over previous
"""Trainium2 Bass kernel: FADEv4 retrieval-kNN head (nn_FADEv4_7026566496861).

Math (per image n):
    cls  = l2norm(mean_s(x_support_cls[n]))          # [1,D]
    q    = l2norm(x_query[n])                        # [Tq,D]
    s    = l2norm(x_support[n])                      # [Ts,D]
    sim  = q @ s.T                                   # [Tq,Ts]
    dmin = 1 - max_ts(sim); idx = argmax_ts(sim)
    pred = sigmoid(q@W1 + s[idx]@W2 + cls@W3 + b)
    out0 = (pred*dmin).reshape(N,1,37,37); out1 = pred.reshape(N,1,37,37)

Sharding: data-parallel over N=16 images -> 8 cores x 2 images, no collectives.

v2 design (vs v1 which used PE transposes + bf16 matmuls):
  * Normalization pipeline per 128-token tile:
      ACT Square+accum (ssq) -> ACT Sqrt(scale=1/768) (denom = |x|*sqrt(1/768))
      -> GPSIMD normalize_recip (divide + cast to bf16; so normalized rows are
      sqrt(768)*x_hat) -> XBAR dma_start_transpose (bf16, [tok,768]->[128,6,tok])
      -> ACT copy-cast bf16->fp8e4.
    No PE transposes, no PSUM->SBUF copies.
  * Big matmul in fp8e4 with MatmulPerfMode.DoubleRow (2 contraction chunks
    per instruction, 2x PE throughput): sim_psum = 768 * cosine.
  * W1 appended as an extra support column (p1 = 768*q_hat@W1 falls out of the
    matmul); W2 as an extra query column (p2 row staged to DRAM, gathered per
    query row by indirect DMA).
  * Supports processed in NG=3 column groups (2048/2048/1408 incl W1) with
    [128,2048] f32 PSUM tiles (4 banks x 2 bufs); DVE max8 + max_index per
    (m-block, group).
  * Combine/argmax/head fully batched per image ([128, MB]-shaped ops).
"""

import math
import os

import numpy as np

import concourse.bass as bass
import concourse.mybir as mybir
import concourse.tile as tile
from concourse import bacc
from concourse.bass import ds, IndirectOffsetOnAxis
from concourse.bass_utils import run_bass_kernel_spmd

F32 = mybir.dt.float32
BF16 = mybir.dt.bfloat16
FP8 = mybir.dt.float8e4
U32 = mybir.dt.uint32
AX = mybir.AxisListType
OP = mybir.AluOpType
ACTF = mybir.ActivationFunctionType
PERF = mybir.MatmulPerfMode

N_FULL, TQ, TS, S, D = 16, 1369, 5476, 4, 768
SIDE = 37
KC = D // 128                       # 6 contraction chunks of 128
SSCALE = float(np.float32(1.0 / 768.0))   # sim_psum = cos / SSCALE
WSCALE = float(1.0 / math.sqrt(1.0 / 768.0))  # scale for W1/W2 columns
W2COL = 1376                        # W2 column in qT
TQE = W2COL + 1                     # 1377 qT columns
QT_PITCH = 1392                     # qTq k-chunk pitch; %16==0 for dual-fp8 LDW
MB = (TQE + 127) // 128             # 11 m-blocks (last: 97 cols, 89 real)
GW = 2048                           # support group width (4 PSUM banks)
NG = 3                              # groups: 2048, 2048, 1408
G2W = 1408                          # g2 sTq extent (512+512+384 MM chunks)
G2REAL = 1380                       # real supports in g2; W1 at col 1380
G2TB = 1392                         # extent written into sTb for g2
QTB_W = 1376                        # qTb extent (1280 + 96 padded edge)
P2D_LEN = TS                        # staged p2 covers exactly the real supports
NEG = -1.0e30
W_ALL = MB * NG * 8                 # 264 combine width

N_CORES = 8
PER_CORE = N_FULL // N_CORES

USE_DR = os.environ.get("FADE_DR", "1") == "1"   # DoubleRow fp8; else bf16
MM_DT = FP8 if USE_DR else BF16


def q_tiles():
    t = [(i * 128, 128, 128) for i in range(10)]
    t.append((1280, 89, 96))
    return t


def s_tiles(g):
    base = g * GW
    if g < 2:
        return [(base + i * 128, 128, 128) for i in range(16)]
    t = [(base + i * 128, 128, 128) for i in range(10)]
    t.append((base + 1280, 100, 112))
    return t


def _mm_chunks(g):
    if g < NG - 1:
        return [(0, 512), (512, 512), (1024, 512), (1536, 512)]
    return [(0, 512), (512, 512), (1024, 384)]


def _emit_image(nc, pools, consts, aps, n, stage=99):
    (img_pool, nmb_pool, stb_pool, stq_pool, scratch, cls_pool, p2_pool,
     psum_mm) = pools
    (w1f, w2f, w3, bh, cbase_f) = consts
    (x_query, x_support, x_cls, p2d_list, c3d_list, out0, out1) = aps

    # ---- cls head scalar: c3b = (cls_hat . W3) + b, broadcast to [128,1] ----
    clsbig = cls_pool.tile([1, S * D], F32, tag="clsbig")
    nc.sync.dma_start(out=clsbig[:, :], in_=x_cls[n])
    clsum = cls_pool.tile([1, D], F32, tag="clsum")
    nc.vector.tensor_add(clsum[:, :], clsbig[:, 0:D], clsbig[:, D:2 * D])
    nc.vector.tensor_add(clsum[:, :], clsum[:, :], clsbig[:, 2 * D:3 * D])
    nc.vector.tensor_add(clsum[:, :], clsum[:, :], clsbig[:, 3 * D:4 * D])
    sc3 = cls_pool.tile([1, D], F32, tag="sc3")
    ss3 = cls_pool.tile([1, 8], F32, tag="ss3")
    nc.vector.tensor_mul(sc3[:, :], clsum[:, :], clsum[:, :])
    nc.vector.tensor_reduce(out=ss3[:, 0:1], in_=sc3[:, :], axis=AX.X, op=OP.add)
    nc.vector.tensor_mul(sc3[:, :], clsum[:, :], w3[:, :])
    nc.vector.tensor_reduce(out=ss3[:, 1:2], in_=sc3[:, :], axis=AX.X, op=OP.add)
    nc.scalar.sqrt(ss3[:, 2:3], ss3[:, 0:1])
    nc.vector.reciprocal(ss3[:, 3:4], ss3[:, 2:3])
    nc.vector.tensor_mul(ss3[:, 4:5], ss3[:, 1:2], ss3[:, 3:4])
    nc.vector.tensor_add(ss3[:, 5:6], ss3[:, 4:5], bh[:, 0:1])
    nc.sync.dma_start(out=c3d_list[n][:, :], in_=ss3[0:1, 5:6])
    c3b = img_pool.tile([128, 1], F32, tag="c3b")
    nc.sync.dma_start(out=c3b[:, :], in_=c3d_list[n][:, :].to_broadcast((128, 1)))

    # ---- normalize a list of token tiles, then XBAR-transpose them ----
    # Phase-split so no hwdge queue head-blocks on the gpsimd divide: all
    # loads + norm passes first (norms alternate ACT/DVE), then all
    # transposes issued back-to-back on the Sync queue.
    def build_tiles(src_row0, tiles, dstb, base):
        # Per pair of tiles: one DMA load, two norm passes (DVE+ACT), one
        # sqrt, two gpsimd divides. raw lifetime stays one pair so the
        # scratch pool never over-commits. Transposes are issued at the end,
        # back-to-back on the Sync hwdge queue (nmb pool holds the group).
        nmbs = []

        use_ttr = os.environ.get("FADE_TTR", "0") == "1"

        def norm_pair(pairs):
            for j, (rawap, rp0) in enumerate(pairs):
                sden = scratch.tile([128, 1], F32, tag="sden")
                if rp0 < 128:
                    nc.vector.memset(sden[:, :], 1.0)
                if use_ttr and j == 0:
                    sqv = scratch.tile([128, D], BF16, tag="sqv")
                    nc.vector.tensor_tensor_reduce(
                        out=sqv[:rp0, :], in0=rawap[:rp0, :],
                        in1=rawap[:rp0, :], scale=1.0, scalar=0.0,
                        op0=OP.mult, op1=OP.add,
                        accum_out=sden[:rp0, :],
                    )
                else:
                    sqa = scratch.tile([128, D], BF16, tag="sqa")
                    nc.scalar.activation(
                        sqa[:rp0, :], rawap[:rp0, :], ACTF.Square,
                        accum_out=sden[:rp0, :],
                    )
                sdq = scratch.tile([128, 1], F32, tag="sdq")
                nc.scalar.activation(sdq[:, :], sden[:, :], ACTF.Sqrt,
                                     scale=SSCALE)
                nmb = nmb_pool.tile([128, D], BF16, tag="nmb")
                nc.gpsimd.normalize_recip(nmb[:rp0, :], rawap[:rp0, :],
                                          sdq[:rp0, :])
                nmbs.append(nmb)

        # Emit transposes one pair BEHIND the norm chain so the hwdge queue
        # never head-blocks waiting on an in-flight gpsimd divide.
        tp_done = [0]

        def issue_tps(upto):
            while tp_done[0] < upto:
                j = tp_done[0]
                tok0, _, trp = tiles[j]
                eng = nc.sync if (j % 2 == 0) else nc.scalar
                eng.dma_start_transpose(
                    out=dstb[:, :, ds(tok0 - base, trp)], in_=nmbs[j][:trp, :]
                )
                tp_done[0] += 1

        i = 0
        while i < len(tiles):
            t0, r0, rp0 = tiles[i]
            if i + 1 < len(tiles) and r0 == 128 and tiles[i + 1][1] == 128:
                raw2 = scratch.tile([128, 2, D], F32, tag="raw2")
                nc.sync.dma_start(
                    out=raw2[:, :, :],
                    in_=src_row0[ds(t0, 256), :].rearrange(
                        "(i p) d -> p i d", p=128),
                )
                norm_pair([(raw2[:, 0, :], 128), (raw2[:, 1, :], 128)])
                i += 2
            else:
                raw = scratch.tile([128, D], F32, tag="raw")
                if r0 < rp0:
                    nc.vector.memset(raw[:, :], 1.0)
                nc.sync.dma_start(out=raw[:r0, :], in_=src_row0[ds(t0, r0), :])
                norm_pair([(raw[:, :], rp0)])
                i += 1
            issue_tps(len(nmbs) - 2)
        issue_tps(len(tiles))

    # ---- build qT (bf16 staged, then cast to fp8) ----
    qTb = stb_pool.tile([128, KC, GW], BF16, tag="sTb")
    build_tiles(x_query[n], q_tiles(), qTb, 0)
    qTq = img_pool.tile([128, KC, QT_PITCH], MM_DT, tag="qTq")
    for (c0, c1) in ((0, 768), (768, QTB_W)):
        nc.vector.tensor_copy(qTq[:, :, c0:c1], qTb[:, :, c0:c1])
    nc.vector.memset(qTq[:, :, TQ:W2COL], 0)
    nc.vector.tensor_copy(
        qTq[:, :, W2COL:W2COL + 1].rearrange("p a b -> p (a b)"), w2f[:, :]
    )

    if stage < 2:
        z0 = scratch.tile([128, MB], F32, tag="z0")
        nc.vector.memset(z0[:, :], 0.25)
        for m in range(MB):
            mreal = 128 if m < MB - 1 else TQ - 128 * (MB - 1)
            nc.sync.dma_start(out=out1[n, ds(m * 128, mreal)], in_=z0[:mreal, m:m + 1])
            nc.sync.dma_start(out=out0[n, ds(m * 128, mreal)], in_=z0[:mreal, m:m + 1])
        return

    # ---- per-image state ----
    Mc8 = img_pool.tile([128, MB, NG, 8], F32, tag="Mc8")
    Ic8 = img_pool.tile([128, MB, NG, 8], U32, tag="Ic8")
    p1_all = img_pool.tile([128, MB], F32, tag="p1")
    # rows >= 97 of the last m-block are never written by max8; give them
    # defined values so the batched combine stays finite (clamp handles the
    # all-equal one-hot sum).
    nc.vector.memset(Mc8[:, :, :, :].rearrange("p a b c -> p (a b c)"), NEG)
    nc.vector.memset(Ic8[:, :, :, :].rearrange("p a b c -> p (a b c)"), 0)
    nc.vector.memset(p1_all[:, :], 0)

    def build_group(g):
        sTb = stb_pool.tile([128, KC, GW], BF16, tag="sTb")
        build_tiles(x_support[n], s_tiles(g), sTb, g * GW)
        sTq = stq_pool.tile([128, KC, GW], MM_DT, tag="sTq")
        if g < NG - 1:
            for (c0, c1) in ((0, 1024), (1024, 2048)):
                nc.vector.tensor_copy(sTq[:, :, c0:c1], sTb[:, :, c0:c1])
        else:
            for (c0, c1) in ((0, 1024), (1024, G2TB)):
                nc.vector.tensor_copy(sTq[:, :, c0:c1], sTb[:, :, c0:c1])
            nc.vector.memset(sTq[:, :, G2TB:G2W], 0)
            nc.vector.tensor_copy(
                sTq[:, :, G2REAL:G2REAL + 1].rearrange("p a b -> p (a b)"),
                w1f[:, :],
            )
        return sTq

    def mm_group(g, sTq):
        gn = GW if g < NG - 1 else G2REAL
        for m in range(MB):
            mcols = 128 if m < MB - 1 else TQE - 128 * (MB - 1)
            bp = psum_mm.tile([128, GW], F32, tag="bp")
            if USE_DR:
                for (c0, cw) in _mm_chunks(g):
                    for k2 in range(KC // 2):
                        nc.tensor.matmul(
                            bp[:mcols, ds(c0, cw)],
                            lhsT=qTq[:, 2 * k2:2 * k2 + 2, ds(m * 128, mcols)],
                            rhs=sTq[:, 2 * k2:2 * k2 + 2, ds(c0, cw)],
                            start=(k2 == 0), stop=(k2 == KC // 2 - 1),
                            perf_mode=PERF.DoubleRow,
                        )
            else:
                for (c0, cw) in _mm_chunks(g):
                    for k in range(KC):
                        nc.tensor.matmul(
                            bp[:mcols, ds(c0, cw)],
                            lhsT=qTq[:, k, ds(m * 128, mcols)],
                            rhs=sTq[:, k, ds(c0, cw)],
                            start=(k == 0), stop=(k == KC - 1),
                        )
            nc.vector.max(Mc8[:mcols, m, g, :], bp[:mcols, :gn])
            nc.vector.max_index(
                Ic8[:mcols, m, g, :], Mc8[:mcols, m, g, :], bp[:mcols, :gn]
            )
            if g == NG - 1:
                nc.scalar.copy(p1_all[:mcols, m:m + 1],
                               bp[:mcols, G2REAL:G2REAL + 1])
            if m == MB - 1:
                p2row = p2_pool.tile([1, GW], F32, tag="p2row")
                nc.scalar.copy(p2row[0:1, :gn], bp[96:97, :gn])
                nc.sync.dma_start(
                    out=p2d_list[n][ds(g * GW, gn), 0], in_=p2row[0:1, :gn]
                )

    # software-pipelined: build group g+1 before the matmuls of group g
    sT0 = build_group(0)
    sT1 = build_group(1)
    mm_group(0, sT0)
    sT2 = build_group(2)
    mm_group(1, sT1)
    mm_group(2, sT2)

    if stage < 3:
        z0 = scratch.tile([128, MB], F32, tag="z0")
        nc.vector.tensor_copy(z0[:, :], Mc8[:, :, 0, 0])
        for m in range(MB):
            mreal = 128 if m < MB - 1 else TQ - 128 * (MB - 1)
            nc.sync.dma_start(out=out1[n, ds(m * 128, mreal)], in_=z0[:mreal, m:m + 1])
            nc.sync.dma_start(out=out0[n, ds(m * 128, mreal)], in_=p1_all[:mreal, m:m + 1])
        return

    # ---- batched combine: global max / argmax per m-block ----
    mcA = Mc8[:, :, :, :].rearrange("p a b c -> p a (b c)")   # [128, MB, 24]
    icA = Ic8[:, :, :, :].rearrange("p a b c -> p (a b c)")   # [128, 264]
    gmax = img_pool.tile([128, MB], F32, tag="gmax")
    nc.vector.tensor_reduce(out=gmax[:, :], in_=mcA, axis=AX.X, op=OP.max)
    oneh = img_pool.tile([128, MB, NG * 8], F32, tag="oneh")
    for m in range(MB):
        nc.vector.tensor_scalar(
            out=oneh[:, m, :], in0=mcA[:, m, :], scalar1=gmax[:, m:m + 1],
            scalar2=None, op0=OP.is_equal,
        )
    icf = img_pool.tile([128, MB, NG * 8], F32, tag="icf")
    icfF = icf[:, :, :].rearrange("p a b -> p (a b)")
    nc.vector.tensor_copy(icfF, icA)
    nc.vector.tensor_add(icfF, icfF, cbase_f[:, :])
    nc.vector.tensor_mul(icfF, icfF,
                         oneh[:, :, :].rearrange("p a b -> p (a b)"))
    gidxf = img_pool.tile([128, MB], F32, tag="gidxf")
    nc.vector.tensor_reduce(
        out=gidxf[:, :], in_=icf[:, :, :], axis=AX.X, op=OP.add,
    )
    nc.vector.tensor_scalar_min(gidxf[:, :], gidxf[:, :], float(TS - 1))
    gidx = img_pool.tile([128, MB], U32, tag="gidx")
    nc.vector.tensor_copy(gidx[:, :], gidxf[:, :])
    dmin_all = img_pool.tile([128, MB], F32, tag="dmin")
    nc.scalar.activation(dmin_all[:, :], gmax[:, :], ACTF.Copy,
                         bias=1.0, scale=-SSCALE)

    # ---- p2 gather + batched head ----
    p2g = img_pool.tile([128, MB], F32, tag="p2g")
    for m in range(MB):
        nc.gpsimd.indirect_dma_start(
            out=p2g[:, m:m + 1], out_offset=None, in_=p2d_list[n][:, :],
            in_offset=IndirectOffsetOnAxis(ap=gidx[:, m:m + 1], axis=0),
        )
    lg = img_pool.tile([128, MB], F32, tag="lg")
    nc.vector.tensor_add(lg[:, :], p1_all[:, :], p2g[:, :])
    pred = img_pool.tile([128, MB], F32, tag="pred")
    nc.scalar.activation(pred[:, :], lg[:, :], ACTF.Sigmoid,
                         bias=c3b[:, 0:1], scale=SSCALE)
    o0 = img_pool.tile([128, MB], F32, tag="o0")
    nc.vector.tensor_mul(o0[:, :], pred[:, :], dmin_all[:, :])
    for m in range(MB):
        mreal = 128 if m < MB - 1 else TQ - 128 * (MB - 1)
        nc.sync.dma_start(out=out1[n, ds(m * 128, mreal)], in_=pred[:mreal, m:m + 1])
        nc.sync.dma_start(out=out0[n, ds(m * 128, mreal)], in_=o0[:mreal, m:m + 1])


def build_program(per_core=PER_CORE, stage=99):
    nc = bacc.Bacc("TRN2", target_bir_lowering=False, debug=False)
    x_query = nc.dram_tensor("x_query", [per_core, TQ, D], F32, kind="ExternalInput").ap()
    x_support = nc.dram_tensor("x_support", [per_core, TS, D], F32, kind="ExternalInput").ap()
    x_cls = nc.dram_tensor("x_support_cls", [per_core, S * D], F32, kind="ExternalInput").ap()
    w_head = nc.dram_tensor("W_head", [3 * D, 1], F32, kind="ExternalInput").ap()
    b_head = nc.dram_tensor("b_head", [1, 1], F32, kind="ExternalInput").ap()
    out0 = nc.dram_tensor("out0", [per_core, TQ], F32, kind="ExternalOutput").ap()
    out1 = nc.dram_tensor("out1", [per_core, TQ], F32, kind="ExternalOutput").ap()
    p2d_list = [
        nc.dram_tensor(f"p2d_{n}", [P2D_LEN, 1], F32).ap() for n in range(per_core)
    ]
    c3d_list = [
        nc.dram_tensor(f"c3d_{n}", [1, 1], F32).ap() for n in range(per_core)
    ]

    with tile.TileContext(nc) as tc:
        with tc.tile_pool(name="img", bufs=2) as img_pool, \
             tc.tile_pool(name="nmbp", bufs=18) as nmb_pool, \
             tc.tile_pool(name="stb", bufs=2) as stb_pool, \
             tc.tile_pool(name="stq", bufs=2) as stq_pool, \
             tc.tile_pool(name="scratch", bufs=4) as scratch, \
             tc.tile_pool(name="cls", bufs=1) as cls_pool, \
             tc.tile_pool(name="p2", bufs=1) as p2_pool, \
             tc.tile_pool(name="const", bufs=1) as const_pool, \
             tc.tile_pool(name="psum_mm", bufs=2, space="PSUM") as psum_mm:

            # constants
            w1s = const_pool.tile([128, KC], F32)
            w2s = const_pool.tile([128, KC], F32)
            w3 = const_pool.tile([1, D], F32)
            bh = const_pool.tile([1, 1], F32)
            for k in range(KC):
                nc.sync.dma_start(out=w1s[:, k:k + 1], in_=w_head[ds(128 * k, 128), :])
                nc.sync.dma_start(out=w2s[:, k:k + 1], in_=w_head[ds(D + 128 * k, 128), :])
            nc.sync.dma_start(out=w3[0:1, :], in_=w_head[ds(2 * D, D), :])
            nc.sync.dma_start(out=bh[:, :], in_=b_head[:, :])
            w1f = const_pool.tile([128, KC], MM_DT)
            w2f = const_pool.tile([128, KC], MM_DT)
            nc.scalar.activation(w1f[:, :], w1s[:, :], ACTF.Copy, scale=WSCALE)
            nc.scalar.activation(w2f[:, :], w2s[:, :], ACTF.Copy, scale=WSCALE)
            cbase_u = const_pool.tile([128, W_ALL], U32)
            cbase_f = const_pool.tile([128, W_ALL], F32)
            nc.gpsimd.iota(cbase_u[:, :], pattern=[[0, MB], [GW, NG], [0, 8]],
                           base=0, channel_multiplier=0)
            nc.vector.tensor_copy(cbase_f[:, :], cbase_u[:, :])

            pools = (img_pool, nmb_pool, stb_pool, stq_pool, scratch, cls_pool,
                     p2_pool, psum_mm)
            consts = (w1f, w2f, w3, bh, cbase_f)
            aps = (x_query, x_support, x_cls, p2d_list, c3d_list, out0, out1)
            for n in range(per_core):
                _emit_image(nc, pools, consts, aps, n, stage=stage)

    nc.compile()
    return nc


_CACHED = {}


def _get_program(per_core=PER_CORE):
    key = (per_core, MM_DT)
    if key not in _CACHED:
        _CACHED[key] = build_program(per_core)
    return _CACHED[key]


def run(inputs, trace=False, per_core=PER_CORE):
    nc = _get_program(per_core)
    n_cores = N_FULL // per_core
    xq = np.ascontiguousarray(inputs["x_query"], dtype=np.float32)
    xs = np.ascontiguousarray(inputs["x_support"], dtype=np.float32)
    xc = np.ascontiguousarray(inputs["x_support_cls"], dtype=np.float32).reshape(
        N_FULL, S * D
    )
    wh = np.ascontiguousarray(inputs["W_head"], dtype=np.float32).reshape(3 * D, 1)
    bhv = np.ascontiguousarray(inputs["b_head"], dtype=np.float32).reshape(1, 1)
    in_maps = []
    for c in range(n_cores):
        sl = slice(c * per_core, (c + 1) * per_core)
        in_maps.append({
            "x_query": xq[sl], "x_support": xs[sl], "x_support_cls": xc[sl],
            "W_head": wh, "b_head": bhv,
        })
    res = run_bass_kernel_spmd(nc, in_maps, list(range(n_cores)), trace=trace)
    o0 = np.concatenate([res.results[c]["out0"] for c in range(n_cores)], axis=0)
    o1 = np.concatenate([res.results[c]["out1"] for c in range(n_cores)], axis=0)
    o0 = o0.reshape(N_FULL, 1, SIDE, SIDE).astype(np.float32)
    o1 = o1.reshape(N_FULL, 1, SIDE, SIDE).astype(np.float32)
    return (o0, o1), res


def kernel(**inputs):
    (o0, o1), _ = run(inputs, trace=False)
    return o0, o1
